# revision 1
# baseline (speedup 1.0000x reference)
"""GCNConv + PReLU on Trainium2, 8-core SPMD Bass/Tile kernel (v2).

Math (PyG GCNConv, add_self_loops=True, symmetric norm):
    h = x @ W
    deg[c] = (# edges with col == c) + 1          (self-loop)
    dis = rsqrt(deg)
    out[c] = dis[c] * ( sum_{e: col_e == c} dis[row_e] * h[row_e] )
             + dis[c]^2 * h[c] + bias             (self-loop term)
    z = max(out, prelu_a * out)                   (PReLU, 0 <= a <= 1)

Distribution: destination nodes sharded across 8 cores (12500 each);
x/W replicated; each core builds the full g = dis*h table locally so
the per-edge source gather is local. Degrees are an integer histogram
of edge_index and are computed host-side (index-only prep, like the
edge sort); the device does the rsqrt and all float math on x/W.

Device pipeline per core:
  B) g table: h = x@W tile-by-tile in bf16 (x host-transposed/cast),
     row-scaled by dis on the Scalar engine, written to 4 DRAM
     sub-tables in partition-major layout (2KB contiguous lines).
  Bown) own-shard recompute of t_w = dis^2*h + bias (self-loop+bias).
  C) per 128-edge tile: bulk dma_gather of source g rows (int16 idx,
     sub-table chunked) + a PE matmul onehotT @ gathered accumulating
     [128 dest, 128 feat] sums in PSUM. One-hot tiles are precomputed
     host-side as fp8 (exact 0/1) and DMA-streamed as matmul lhsT --
     no per-edge DVE work at all. Windows processed in groups of 28
     (7 PSUM banks x 4 windows/bank) held PSUM-resident across the 4
     sub-table rounds; flush = scale+t_w+PReLU and DMA out.
"""

import math
import sys

for _p in ("/opt/trn_rl_repo",):
    if _p not in sys.path:
        sys.path.insert(0, _p)

import numpy as np
import ml_dtypes

P = 128
CORES = 8
NSUB = 4
GCT = 32  # tiles per dma_gather chunk
XCH = 1024  # x columns per load
GB = 8  # g row-tiles staged per DMA write (= XCH // P)

FULL_CFG = dict(N=100000, F_IN=256, F_OUT=128, E=1600000)

_prog_cache = {}


def _derived(cfg):
    N = cfg["N"]
    shard = N // CORES
    nw = math.ceil(shard / P)  # windows per shard
    NT = math.ceil(N / P)  # global row-tiles
    NP = NT * P
    # sub-table sizes in row-tiles: under the int16 row-index limit,
    # divisible by the 4-tile phase-B write batch. Unequal split chosen
    # so per-(window, sub) edge-bucket means land just below a 128-edge
    # tile boundary (minimizes ceil+max-over-cores padding).
    if NT == 784:
        ch_t = [224, 224, 224, 112]
    else:
        base = min(248, -(-math.ceil(NT / NSUB) // 8) * 8)
        ch_t = []
        rem = NT
        for s in range(NSUB):
            t = min(base, rem)
            ch_t.append(t)
            rem -= t
        assert rem == 0
    assert sum(ch_t) == NT and all(t * P <= 32767 or t == 0 for t in ch_t)
    jb = np.concatenate([[0], np.cumsum(ch_t)])  # row-tile bases
    return shard, nw, NT, NP, ch_t, jb


def _schedule(tsw, nw):
    """Segment layout: one contiguous tile run per sub-table s
    (w-ascending within), padded to a GCT multiple. Returns per-(w,s)
    tile base offsets, segment table, and total padded tile count T."""
    tile_base = np.zeros((nw, NSUB), np.int64)
    segs = []  # (start, real, padded) per s
    pos = 0
    for s in range(NSUB):
        start = pos
        for w in range(nw):
            tile_base[w, s] = pos
            pos += tsw[w][s]
        real = pos - start
        padded = -(-real // GCT) * GCT
        segs.append((start, real, padded))
        pos = start + padded
    return tile_base, segs, pos


def host_prep(x, edge_index, W, bias, prelu_a, cfg):
    """Index/layout prep: integer degree histogram, shard + sort edges by
    (dest window, source sub-table), fixed 128-edge tiles, int16 wrapped
    gather indices (partition-major g layout), fp8 one-hot tiles, and
    bf16 cast/transpose of x. All float math on x/W stays on device."""
    N, F_IN, F_OUT = cfg["N"], cfg["F_IN"], cfg["F_OUT"]
    shard, nw, NT, NP, ch_t, jb = _derived(cfg)

    x = np.asarray(x, np.float32)
    W = np.asarray(W, np.float32)
    bias = np.asarray(bias, np.float32)
    prelu_a = np.asarray(prelu_a, np.float32)
    ei = np.asarray(edge_index)

    rows_all = np.asarray(ei[0]).astype(np.int64)
    cols_all = np.asarray(ei[1]).astype(np.int64)

    # integer degree histogram (+1 self-loop), laid out for the device
    deg = np.bincount(cols_all, minlength=N).astype(np.float32) + 1.0
    deg_all = np.ones(NP, np.float32)
    deg_all[:N] = deg
    deg_all = np.ascontiguousarray(deg_all.reshape(NT, P).T)  # [P, NT]
    deg_own = np.ones((CORES, nw * P), np.float32)
    for k in range(CORES):
        deg_own[k, :shard] = deg[k * shard : (k + 1) * shard]
    deg_own = np.ascontiguousarray(
        deg_own.reshape(CORES, nw, P).transpose(0, 2, 1)
    )  # [CORES, P, nw]

    order = np.argsort(cols_all, kind="stable")
    rs = rows_all[order]
    cs = cols_all[order]
    bounds = np.searchsorted(cs, np.arange(CORES + 1) * shard)

    # source row r -> sub-table s and in-table position (partition-major:
    # row r = 128j+p stored at p*ch_t[s] + (j - jb[s]))
    cnts = np.zeros((CORES, nw, NSUB), np.int64)
    per_core = []
    jb_rows = jb * P
    for k in range(CORES):
        seg = slice(bounds[k], bounds[k + 1])
        local = cs[seg] - k * shard
        w_arr = local // P
        r_arr = rs[seg]
        j_arr = r_arr // P
        s_arr = np.searchsorted(jb[1:], j_arr, side="right")
        key = w_arr * NSUB + s_arr
        o2 = np.argsort(key, kind="stable")
        cnts[k] = np.bincount(key, minlength=nw * NSUB).reshape(nw, NSUB)
        per_core.append((local[o2], w_arr[o2], s_arr[o2], r_arr[o2], key[o2]))

    tsw = (-(-cnts // P)).max(axis=0)  # [nw, NSUB]
    tsw[:, 0] = np.maximum(tsw[:, 0], 1)  # every window needs >= 1 matmul
    tsw_l = tuple(tuple(int(v) for v in row) for row in tsw)
    tile_base, segs, T = _schedule(tsw_l, nw)

    idx16 = np.zeros((CORES, 16, T * 8), np.int16)
    oh8 = np.zeros((CORES, P, T * P), ml_dtypes.float8_e4m3)
    for k in range(CORES):
        local, w_arr, s_arr, r_arr, key = per_core[k]
        cnt_flat = cnts[k].reshape(-1)
        gstart = np.concatenate([[0], np.cumsum(cnt_flat)])
        within = np.arange(local.size) - gstart[key]
        slot = tile_base[w_arr, s_arr] * P + within
        p_arr = slot % P
        t_arr = slot // P
        # gather position: partition-major within sub-table s
        ct = np.asarray(ch_t)[s_arr]
        pos = (r_arr % P) * ct + (r_arr // P - np.asarray(jb)[s_arr])
        idx16[k, p_arr % 16, t_arr * 8 + p_arr // 16] = pos.astype(np.int16)
        oh8[k, p_arr, t_arr * P + (local - w_arr * P)] = 1.0
        # trailing pad tiles of each segment: idx = -1 (descriptors skipped)
        for start, real, padded in segs:
            if padded > real:
                idx16[k, :, (start + real) * 8 : (start + padded) * 8] = -1
    idx16_rep = np.ascontiguousarray(np.tile(idx16, (1, P // 16, 1)))

    xp = np.zeros((NP, F_IN), np.float32)
    xp[:N] = x
    x_t = np.ascontiguousarray(xp.T.astype(ml_dtypes.bfloat16))  # [F_IN, NP]

    return dict(
        tsw=tsw_l,
        T=T,
        x_t=x_t,
        w=np.ascontiguousarray(W.astype(ml_dtypes.bfloat16)),
        bias_b=np.ascontiguousarray(np.tile(bias[None, :], (P, 1))),
        prelu_b=np.ascontiguousarray(np.tile(prelu_a[None, :], (P, 1))),
        deg_all=deg_all,
        deg_own=deg_own,
        idx16=idx16_rep,
        oh8=oh8,
    )


def build_program(cfg, tsw, debug_outs=False):
    import concourse.bass as bass
    import concourse.bacc as bacc
    import concourse.mybir as mybir
    import concourse.tile as tile
    from concourse.bass import ds

    f32 = mybir.dt.float32
    bf16 = mybir.dt.bfloat16
    fp8 = mybir.dt.float8e4
    i16 = mybir.dt.int16
    AOT = mybir.AluOpType
    ACT = mybir.ActivationFunctionType

    N, F_IN, F_OUT = cfg["N"], cfg["F_IN"], cfg["F_OUT"]
    shard, nw, NT, NP, ch_t, jb = _derived(cfg)
    kchunks = F_IN // P
    tile_base, segs, T = _schedule(tsw, nw)

    nc = bacc.Bacc(
        "TRN2",
        target_bir_lowering=False,
        debug=False,
        num_devices=CORES,
        num_swdge_queues=4,
    )

    x_t = nc.dram_tensor("x_t", [F_IN, NP], bf16, kind="ExternalInput")
    w_d = nc.dram_tensor("w", [F_IN, F_OUT], bf16, kind="ExternalInput")
    bias_d = nc.dram_tensor("bias_b", [P, F_OUT], f32, kind="ExternalInput")
    prelu_d = nc.dram_tensor("prelu_b", [P, F_OUT], f32, kind="ExternalInput")
    dega_d = nc.dram_tensor("deg_all", [P, NT], f32, kind="ExternalInput")
    dego_d = nc.dram_tensor("deg_own", [P, nw], f32, kind="ExternalInput")
    idx16_d = nc.dram_tensor("idx16", [P, T * 8], i16, kind="ExternalInput")
    oh_d = nc.dram_tensor("oh8", [P, T * P], fp8, kind="ExternalInput")
    out_d = nc.dram_tensor("out", [shard, F_OUT], f32, kind="ExternalOutput")

    g_subs = [
        nc.dram_tensor(f"g_sub{s}", [ch_t[s] * P, F_OUT], bf16) for s in range(NSUB)
    ]

    with tile.TileContext(nc, pool_alloc_mode="queue") as tc:
        with (
            tc.tile_pool(name="const", bufs=1) as constp,
            tc.tile_pool(name="dis", bufs=1) as disp,
            tc.tile_pool(name="tw", bufs=1) as twp,
        ):
            wt = []
            for c in range(kchunks):
                wc = constp.tile([P, F_OUT], bf16, tag=f"wc{c}")
                nc.sync.dma_start(out=wc[:], in_=w_d[c * P : (c + 1) * P, :])
                wt.append(wc)
            biasb = constp.tile([P, F_OUT], f32)
            nc.sync.dma_start(out=biasb[:], in_=bias_d[:, :])
            prelub = constp.tile([P, F_OUT], f32)
            nc.sync.dma_start(out=prelub[:], in_=prelu_d[:, :])

            # whole idx array resident in SBUF: one early DMA so gathers
            # never wait behind phase-B's DMA issue streams
            idx_all = disp.tile([P, T * 8], i16, name="idx_all")
            nc.sync.dma_start(out=idx_all[:], in_=idx16_d[:, :])

            dis_a = disp.tile([P, NT], f32)
            nc.sync.dma_start(out=dis_a[:], in_=dega_d[:, :])
            nc.scalar.activation(out=dis_a[:], in_=dis_a[:], func=ACT.Sqrt)
            nc.vector.reciprocal(out=dis_a[:], in_=dis_a[:])
            dis_o = disp.tile([P, nw], f32)
            nc.sync.dma_start(out=dis_o[:], in_=dego_d[:, :])
            nc.scalar.activation(out=dis_o[:], in_=dis_o[:], func=ACT.Sqrt)
            nc.vector.reciprocal(out=dis_o[:], in_=dis_o[:])
            # dis^2 for the self-loop term (keeps phase Bown off the DVE)
            disq = disp.tile([P, nw], f32)
            nc.vector.tensor_tensor(out=disq[:], in0=dis_o[:], in1=dis_o[:], op=AOT.mult)
            zc = disp.tile([P, 1], f32)
            nc.vector.memset(zc[:], 0.0)

            # ---------------- Phase B: g table = dis * (x @ W) ----------
            with (
                nc.named_scope("phaseB"),
                tc.tile_pool(name="b_x", bufs=4) as bxp,
                tc.tile_pool(name="b_ps", bufs=4, space="PSUM") as bpsp,
                tc.tile_pool(name="b_g", bufs=4) as bgp,
            ):
                # x loads on the Scalar engine with explicit lookahead, so
                # the sync engine stays free to stream phase-C's one-hot
                # loads from t~0 (sync head-of-line blocking on the xt buf
                # ring was delaying phase C's matmul inputs by ~300us)
                chunks = list(range(0, NP, XCH))
                LA = 3
                xtiles = {}

                def load_x(ci):
                    if ci >= len(chunks):
                        return
                    c0 = chunks[ci]
                    cl = min(XCH, NP - c0)
                    xt = bxp.tile([P, 2 * XCH], bf16, tag="xt", name="xt")
                    nc.scalar.dma_start(
                        out=xt[:, : 2 * cl].rearrange("p (t c) -> p t c", t=2),
                        in_=x_t[:, c0 : c0 + cl].rearrange(
                            "(t p) c -> p t c", p=P
                        ),
                    )
                    xtiles[ci] = xt

                for ci in range(LA):
                    load_x(ci)
                for ci, c0 in enumerate(chunks):
                    cl = min(XCH, NP - c0)
                    load_x(ci + LA)
                    xt = xtiles.pop(ci)
                    gt = bgp.tile([P, GB * F_OUT], bf16, tag="bg")
                    nt0 = c0 // P
                    ntiles = cl // P
                    assert ntiles <= GB
                    for jj in range(ntiles):
                        j = nt0 + jj
                        ph = bpsp.tile([P, F_OUT], f32, tag="bps")
                        for c in range(kchunks):
                            nc.tensor.matmul(
                                out=ph[:],
                                lhsT=xt[:, c * cl + jj * P : c * cl + (jj + 1) * P],
                                rhs=wt[c][:],
                                start=(c == 0),
                                stop=(c == kchunks - 1),
                            )
                        # row-scale by dis on the Scalar engine (keeps DVE free)
                        nc.scalar.mul(
                            gt[:, jj * F_OUT : (jj + 1) * F_OUT],
                            ph[:],
                            dis_a[:, j : j + 1],
                        )
                    # partition-major table write: row 128j+p at p*ch_t+j-jb
                    s = int(np.searchsorted(jb[1:], nt0, side="right"))
                    assert nt0 + ntiles <= jb[s + 1]
                    nc.scalar.dma_start(
                        out=g_subs[s]
                        .rearrange("(p j) f -> p j f", p=P)[
                            :, nt0 - int(jb[s]) : nt0 - int(jb[s]) + ntiles, :
                        ],
                        in_=gt[:, : ntiles * F_OUT].rearrange(
                            "p (j f) -> p j f", f=F_OUT
                        ),
                    )

            # ------- own-shard t_w = dis^2*h + bias (self-loop + bias) ---
            tw = twp.tile([P, nw * F_OUT], bf16)
            pid = nc.partition_id()
            with (
                nc.named_scope("phaseBown"),
                tc.tile_pool(name="o_x", bufs=4) as oxp,
                tc.tile_pool(name="o_ps", bufs=4, space="PSUM") as opsp,
            ):
                ochunks = list(range(0, shard, XCH))
                oxtiles = {}

                def load_ox(ci):
                    if ci >= len(ochunks):
                        return
                    c0 = ochunks[ci]
                    cl = min(XCH, shard - c0)
                    xts = []
                    for c in range(kchunks):
                        xt = oxp.tile([P, XCH], bf16, tag=f"oxt{c}", name=f"oxt{c}")
                        nc.scalar.dma_start(
                            out=xt[:, :cl],
                            in_=x_t[c * P : (c + 1) * P, ds(pid * shard + c0, cl)],
                        )
                        xts.append(xt)
                    oxtiles[ci] = xts

                for ci in range(2):
                    load_ox(ci)
                for ci, c0 in enumerate(ochunks):
                    cl = min(XCH, shard - c0)
                    load_ox(ci + 2)
                    xts = oxtiles.pop(ci)
                    for s0 in range(0, cl, P):
                        nn = min(P, cl - s0)
                        w = (c0 + s0) // P
                        ph = opsp.tile([P, F_OUT], f32, tag="ops")
                        for c in range(kchunks):
                            nc.tensor.matmul(
                                out=ph[:nn, :],
                                lhsT=xts[c][:, s0 : s0 + nn],
                                rhs=wt[c][:],
                                start=(c == 0),
                                stop=(c == kchunks - 1),
                            )
                        if nn < P:
                            nc.vector.memset(tw[:, w * F_OUT : (w + 1) * F_OUT], 0.0)
                        # t_w = dis^2 * h (bias folded in at flush)
                        nc.scalar.mul(
                            tw[:nn, w * F_OUT : (w + 1) * F_OUT],
                            ph[:nn, :],
                            disq[:nn, w : w + 1],
                        )

            # ---------------- Phase C: gather + scatter matmuls ---------
            # Per-bucket PSUM accumulation groups (a PSUM bank allows only
            # one open group at a time), window sums held in an SBUF
            # accumulator across the 4 sub-table rounds.
            accw = twp.tile([P, nw * F_OUT], f32, name="accw")
            with (
                nc.named_scope("phaseC"),
                tc.tile_pool(name="c_oh", bufs=4) as cohp,
                tc.tile_pool(name="c_g", bufs=5) as cgp,
                tc.tile_pool(name="c_ps", bufs=8, space="PSUM") as cpsp,
                tc.tile_pool(name="c_f", bufs=4) as cfp,
            ):
                slast = [
                    max(s for s in range(NSUB) if (tsw[w][s] > 0 or s == 0))
                    for w in range(nw)
                ]

                def flush(w):
                    nn = min(P, shard - w * P)
                    acc = cfp.tile([P, F_OUT], f32, tag="facc", name="facc")
                    nc.scalar.mul(
                        acc[:],
                        accw[:, w * F_OUT : (w + 1) * F_OUT],
                        dis_o[:, w : w + 1],
                    )
                    nc.vector.tensor_tensor(
                        out=acc[:],
                        in0=acc[:],
                        in1=tw[:, w * F_OUT : (w + 1) * F_OUT],
                        op=AOT.add,
                    )
                    nc.vector.tensor_tensor(
                        out=acc[:], in0=acc[:], in1=biasb[:], op=AOT.add
                    )
                    neg = cfp.tile([P, F_OUT], f32, tag="fneg", name="fneg")
                    nc.vector.tensor_tensor(
                        out=neg[:], in0=acc[:], in1=prelub[:], op=AOT.mult
                    )
                    nc.vector.tensor_tensor(
                        out=acc[:], in0=acc[:], in1=neg[:], op=AOT.max
                    )
                    nc.scalar.dma_start(
                        out=out_d[w * P : w * P + nn, :], in_=acc[:nn, :]
                    )

                gq = 0
                for s in range(NSUB):
                    start, real, padded = segs[s]
                    # tile index within segment -> window, bucket first/last
                    tmap = []
                    bfirst = []
                    blast = []
                    for w in range(nw):
                        nt = tsw[w][s]
                        tmap += [w] * nt
                        bfirst += [True] + [False] * (nt - 1) if nt else []
                        blast += [False] * (nt - 1) + [True] if nt else []
                    cur = -1
                    oht = gch = None
                    ps = None
                    for tg in range(real):
                        ck = tg // GCT
                        if ck != cur:
                            cur = ck
                            ck0 = ck * GCT
                            nreal = min(real - ck0, GCT)
                            t0 = start + ck0
                            oht = cohp.tile([P, GCT * P], fp8, tag="oh", name="oh")
                            nc.sync.dma_start(
                                out=oht[:, : nreal * P],
                                in_=oh_d[:, t0 * P : (t0 + nreal) * P],
                            )
                            gch = cgp.tile([P, GCT * F_OUT], bf16, tag="cg", name="cg")
                            nc.gpsimd.dma_gather(
                                out_ap=gch[:].rearrange("p (n e) -> p n e", e=F_OUT),
                                in_ap=g_subs[s][:, :],
                                idxs_ap=idx_all[:, t0 * 8 : (t0 + GCT) * 8],
                                num_idxs=GCT * P,
                                num_idxs_reg=nreal * P,
                                elem_size=F_OUT,
                                single_packet=False,
                                queue_num=gq % 4,
                            )
                            gq += 1
                        ti = tg - cur * GCT
                        w = tmap[tg]
                        if bfirst[tg]:
                            ps = cpsp.tile([P, F_OUT], f32, tag="cps", name="cps")
                        nc.tensor.matmul(
                            out=ps[:],
                            lhsT=oht[:, ti * P : (ti + 1) * P],
                            rhs=gch[:, ti * F_OUT : (ti + 1) * F_OUT],
                            start=bfirst[tg],
                            stop=blast[tg],
                        )
                        if blast[tg]:
                            if s == 0:
                                nc.vector.tensor_scalar(
                                    out=accw[:, w * F_OUT : (w + 1) * F_OUT],
                                    in0=ps[:],
                                    scalar1=zc[:],
                                    scalar2=None,
                                    op0=AOT.add,
                                )
                            else:
                                nc.vector.tensor_tensor(
                                    out=accw[:, w * F_OUT : (w + 1) * F_OUT],
                                    in0=accw[:, w * F_OUT : (w + 1) * F_OUT],
                                    in1=ps[:],
                                    op=AOT.add,
                                )
                            if s == slast[w]:
                                flush(w)

    nc.compile()
    return nc


def _get_program(cfg, tsw, debug_outs=False):
    key = (tuple(sorted(cfg.items())), tsw, debug_outs)
    if key not in _prog_cache:
        _prog_cache[key] = build_program(cfg, tsw, debug_outs)
    return _prog_cache[key]


def make_in_maps(prep):
    return [
        {
            "x_t": prep["x_t"],
            "w": prep["w"],
            "bias_b": prep["bias_b"],
            "prelu_b": prep["prelu_b"],
            "deg_all": prep["deg_all"],
            "deg_own": prep["deg_own"][k],
            "idx16": prep["idx16"][k],
            "oh8": prep["oh8"][k],
        }
        for k in range(CORES)
    ]


def kernel(x, edge_index, W, bias, prelu_a, cfg=None):
    from concourse import bass_utils

    cfg = cfg or FULL_CFG
    prep = host_prep(x, edge_index, W, bias, prelu_a, cfg)
    nc = _get_program(cfg, prep["tsw"])
    res = bass_utils.run_bass_kernel_spmd(
        nc, make_in_maps(prep), core_ids=list(range(CORES))
    )
    out = np.concatenate([res.results[k]["out"] for k in range(CORES)], axis=0)
    return out.astype(np.float32)

